# revision 12
# baseline (speedup 1.0000x reference)
"""NT-Xent loss Trainium2 kernel, symmetric/circulant variant (8-core SPMD).

sim = Z Z^T is SYMMETRIC: row sums of exp(sim/tau) only need the upper
circulant half. Each core (rotation makes local rows 0..1023 its own)
computes blocks (a, b) for its 8 row-tiles a and b = a+k, k = 0..32 --
tiles 0..39 of the rotated matrix only (2.5MiB in, 40 transposes, and
HALF the exp work of the full-matrix kernel: 264 vs 512 tile-exps).

Per block (a,b): ACT exps the PSUM sim batch into SBUF (bf16) with
accum_out giving this a-row's partial row sums. The PE then multiplies
ones^T @ E (ldweights=ones) accumulating per-COLUMN sums into a PSUM
colacc window per 8-tile octet -- those are the row-sum contributions of
the mirrored blocks (b, a), output per-core as colpart and scatter-added
on the host (np.roll). Blocks at k=32 are computed by BOTH endpoint
cores (the circulant wraps), each contributing row sums only. The k=0
diagonal block's colsums are excluded (its rows are fully in rowp).
Colaccs are DVE-zeroed and accumulated with start=False ONLY: pure
accumulates commute, so the list scheduler cannot corrupt them
(start=True resets interleaved with accumulates DO get misordered).

Host: den_r = rowp_r + colp_r - exp(z2_r/tau);
      loss = mean(log den - pos/tau).
"""

import numpy as np

B = 4096
TB = 2 * B
D = 128
TAU = 0.5
N_CORES = 8
R = TB // N_CORES   # 1024 rows per core
MT = R // 128       # 8 row-tiles owned per core
NT = TB // 128      # 64 row-tiles total
NO = 5              # octets of column tiles held per core (tiles 0..39)
CT = NO * 8         # 40 column tiles
MAGIC = 0x5F3759DF

_CACHE = {}


def _build():
    import concourse.tile as tile
    from concourse import bacc, mybir

    f32 = mybir.dt.float32
    bf16 = mybir.dt.bfloat16
    i32 = mybir.dt.int32
    Exp = mybir.ActivationFunctionType.Exp
    OpAdd = mybir.AluOpType.add
    OpMult = mybir.AluOpType.mult
    OpShr = mybir.AluOpType.arith_shift_right
    OpXor = mybir.AluOpType.bitwise_xor
    AxisX = mybir.AxisListType.X

    nc = bacc.Bacc(
        "TRN2", target_bir_lowering=False, debug=False, num_devices=N_CORES
    )
    # host pre-rotated AND pre-transposed: [128(p), 64(j)*128(d)]
    e_ap = nc.dram_tensor("e", [128, NT * D], bf16, kind="ExternalInput").ap()
    ident_ap = nc.dram_tensor("ident", [128, 128], bf16, kind="ExternalInput").ap()
    ones_ap = nc.dram_tensor("ones", [128, 128], bf16, kind="ExternalInput").ap()
    rs_ap = nc.dram_tensor("rs", [128, MT], f32, kind="ExternalOutput").ap()
    cp_ap = nc.dram_tensor("colp", [1, CT * 128], f32, kind="ExternalOutput").ap()
    pos_ap = nc.dram_tensor("pos", [128, MT], f32, kind="ExternalOutput").ap()
    z2_ap = nc.dram_tensor("z2", [128, MT], f32, kind="ExternalOutput").ap()

    def chunks512(lo, hi):
        # matmul output must not cross a PSUM bank (512 f32 cols)
        j = lo
        while j < hi:
            je = min((j // 512 + 1) * 512, hi)
            yield j, je
            j = je

    with tile.TileContext(nc) as tc:
        with (
            tc.tile_pool(name="xp", bufs=1) as xp,
            tc.tile_pool(name="ztp", bufs=1) as ztp,
            tc.tile_pool(name="small", bufs=1) as sp,
            tc.tile_pool(name="sq", bufs=2) as sqp,
            tc.tile_pool(name="ep", bufs=2) as ep,
            tc.tile_pool(name="ps", bufs=2, space="PSUM") as pp,
            tc.tile_pool(name="ca", bufs=2, space="PSUM") as cap,
        ):
            ident = sp.tile([128, 128], bf16, tag="ident")
            nc.scalar.dma_start(ident[:], ident_ap[:])
            ones = sp.tile([128, 128], bf16, tag="ones")
            nc.scalar.dma_start(ones[:], ones_ap[:])
            dummy = sp.tile([128, 1], f32, tag="dummy")
            nc.scalar.activation(dummy[:], ident[:, 0:1], Exp)

            # Input DMA on the sync queue in octet order; octet 0 split in
            # halves so its prep chain starts ~1us earlier.
            xgs = [
                xp.tile([128, 1024], bf16, tag=f"xg{o}", name=f"xg{o}")
                for o in range(NO)
            ]
            # tiles 4-7 first: o0 blocks [a..7] all need tile 7, and with
            # descending a the first block needs ONLY tile 7.
            nc.sync.dma_start(xgs[0][:, 512:1024], e_ap[:, 512:1024])
            nc.sync.dma_start(xgs[0][:, 0:512], e_ap[:, 0:512])
            for o in range(1, NO):
                nc.sync.dma_start(xgs[o][:], e_ap[:, o * 1024 : (o + 1) * 1024])

            def xtile(t):
                o, j = divmod(t, 8)
                return xgs[o][:, j * 128 : (j + 1) * 128]

            s2 = sp.tile([128, CT], f32, tag="s2")
            inv = sp.tile([128, CT], f32, tag="inv")
            nrt = sp.tile([128, CT], f32, tag="nrt")
            rsparts = sp.tile([128, MT * NO], f32, tag="rsparts")
            rs_t = sp.tile([128, MT], f32, tag="rs")
            pos_t = sp.tile([128, MT], f32, tag="pos")
            z2_t = sp.tile([128, MT], f32, tag="z2")
            inv2 = sp.tile([128, MT], f32, tag="inv2")
            colpart = sp.tile([128, CT * 128], f32, tag="colpart")

            zts = [
                ztp.tile([128, 1024], bf16, tag=f"zt{o}", name=f"zt{o}")
                for o in range(NO)
            ]

            def zttile(t):
                o, j = divmod(t, 8)
                return zts[o][:, j * 128 : (j + 1) * 128]

            def rsqrt(cols):
                s2i = s2[:, cols].bitcast(i32)
                invi = inv[:, cols].bitcast(i32)
                nc.vector.tensor_scalar(
                    out=invi, in0=s2i, scalar1=1, scalar2=-1,
                    op0=OpShr, op1=OpXor,
                )
                nc.vector.tensor_scalar(
                    out=invi, in0=invi, scalar1=MAGIC + 1, scalar2=None, op0=OpAdd
                )
                nr = nrt[:, cols]
                nc.vector.tensor_tensor(nr, inv[:, cols], inv[:, cols], OpMult)
                nc.vector.tensor_tensor(nr, nr, s2[:, cols], OpMult)
                nc.vector.tensor_scalar(
                    out=nr, in0=nr, scalar1=-0.5, scalar2=1.5,
                    op0=OpMult, op1=OpAdd,
                )
                nc.vector.tensor_tensor(inv[:, cols], inv[:, cols], nr, OpMult)

            def prep_dve(o, j0=0, nj=8):
                gcols = slice(o * 8 + j0, o * 8 + j0 + nj)
                sq = sqp.tile([128, nj * 128], f32, tag="sq", name=f"sq{o}_{j0}")
                xs = xgs[o][:, j0 * 128 : (j0 + nj) * 128]
                nc.vector.tensor_tensor(sq[:], xs, xs, OpMult)
                sq3 = sq[:].rearrange("p (j d) -> p j d", d=128)
                nc.vector.tensor_reduce(s2[:, gcols], sq3, axis=AxisX, op=OpAdd)
                rsqrt(gcols)
                for j in range(nj):
                    t = o * 8 + j0 + j
                    nc.vector.tensor_scalar_mul(xtile(t), xtile(t), inv[:, t : t + 1])

            def prep_tp(o, j0, cn, ncopy=1):
                tp = pp.tile([128, 1024], bf16, tag="ps", name=f"tp{o}_{j0}")
                for j in range(cn):
                    nc.tensor.transpose(
                        tp[:, j * 128 : (j + 1) * 128], xtile(o * 8 + j0 + j),
                        ident[:],
                    )
                cc = cn // ncopy
                for k in range(0, cn, cc):
                    nc.vector.tensor_copy(
                        zts[o][:, (j0 + k) * 128 : (j0 + k + cc) * 128],
                        tp[:, k * 128 : (k + cc) * 128],
                    )

            def block(o, a, colacc):
                """Process pair-batch (octet o, row-tile a)."""
                blo = max(a, o * 8)
                bhi = min(a + 32, o * 8 + 7)
                nb = bhi - blo + 1
                c0 = (blo - o * 8) * 128
                cw = nb * 128
                mm = pp.tile([128, 1024], f32, tag="ps", name=f"mm{o}_{a}")
                lhsT = zttile(a)
                for j, je in chunks512(c0, c0 + cw):
                    nc.tensor.matmul(mm[:, j:je], lhsT, zts[o][:, j:je])
                et = ep.tile([128, 1024], bf16, tag="e", name=f"e{o}_{a}")
                nc.scalar.activation(
                    et[:, c0 : c0 + cw], mm[:, c0 : c0 + cw], Exp,
                    scale=1.0 / TAU,
                    accum_out=rsparts[:, a * NO + o : a * NO + o + 1],
                )
                # Excluded from colsums: the k==32 tile (o==4 last tile;
                # row-accum on both endpoint cores) and the k==0 diagonal
                # (o==0 first tile; its colsums equal its rowp entry).
                clo = c0 + 128 if o == 0 else 0
                chi = c0 + cw if o < 4 else a * 128
                for j, je in chunks512(clo, chi):
                    nc.tensor.matmul(
                        colacc[:, j:je], ones[:], et[:, j:je],
                        start=False, stop=True,
                    )

            # scheduling anchors (sim ms) -- SPARSE: only to hold next-octet
            # prep out of the current octet's early window. (Dense anchoring
            # rate-limits the real schedule -- measured, do not do it.)
            OSTART = [0.0045, 0.0115, 0.0215, 0.0315, 0.0405]

            # octet-0 prep in halves (ramp-critical), tiles 4-7 first
            prep_dve(0, 4, 4)
            prep_tp(0, 4, 4, ncopy=2)
            prep_dve(0, 0, 4)
            prep_tp(0, 0, 4, ncopy=2)

            for o in range(NO):
                if o == 1:
                    with tc.tile_wait_until(OSTART[1]):
                        nc.vector.tensor_tensor(
                            inv2[:], inv[:, :MT], inv[:, :MT], OpMult
                        )
                        nc.vector.tensor_tensor(
                            z2_t[:], inv2[:], s2[:, :MT], OpMult
                        )
                if o == 4:
                    with tc.tile_wait_until(OSTART[4]):
                        psq = sqp.tile([128, MT * 128], f32, tag="sq", name="psq")
                        nc.vector.tensor_tensor(
                            psq[:], xgs[0][:], xgs[4][:], OpMult
                        )
                        psq3 = psq[:].rearrange("p (m d) -> p m d", d=128)
                        nc.vector.tensor_reduce(pos_t[:], psq3, axis=AxisX, op=OpAdd)
                if o == 0:
                    colacc = cap.tile([128, 1024], f32, tag="ca", name="ca0")
                    nc.vector.memset(colacc[:], 0.0)
                else:
                    colacc = colacc_next
                aorder = range(MT - 1, -1, -1) if o == 0 else range(MT)
                for ai, a in enumerate(aorder):
                    if o < NO - 1 and ai == 0:
                        with tc.tile_wait_until(OSTART[o]):
                            prep_dve(o + 1)
                    if o < NO - 1 and ai == 5:
                        w = OSTART[o + 1] - OSTART[o]
                        with tc.tile_wait_until(OSTART[o] + 0.55 * w):
                            prep_tp(o + 1, 0, 4)
                        with tc.tile_wait_until(OSTART[o] + 0.72 * w):
                            prep_tp(o + 1, 4, 4)
                    if o < NO - 1 and ai == 6:
                        colacc_next = cap.tile(
                            [128, 1024], f32, tag="ca", name=f"ca{o + 1}"
                        )
                        with tc.tile_wait_until(
                            OSTART[o] + 0.8 * (OSTART[o + 1] - OSTART[o])
                        ):
                            nc.vector.memset(colacc_next[:], 0.0)
                    block(o, a, colacc)
                nc.vector.tensor_copy(
                    colpart[:, o * 1024 : (o + 1) * 1024], colacc[:]
                )

            rsp3 = rsparts[:].rearrange("p (a o) -> p a o", o=NO)
            nc.vector.tensor_reduce(rs_t[:], rsp3, axis=AxisX, op=OpAdd)

            nc.sync.dma_start(rs_ap[:], rs_t[:])
            nc.sync.dma_start(cp_ap[:], colpart[0:1, :])
            nc.sync.dma_start(pos_ap[:], pos_t[:])
            nc.sync.dma_start(z2_ap[:], z2_t[:])

    nc.compile()
    return nc


def _get_nc():
    if "nc" not in _CACHE:
        _CACHE["nc"] = _build()
    return _CACHE["nc"]


def kernel(e_i: np.ndarray, e_j: np.ndarray, _trace: bool = False):
    import ml_dtypes
    from concourse.bass_utils import run_bass_kernel_spmd

    bf16 = ml_dtypes.bfloat16
    nc = _get_nc()
    e = np.concatenate(
        [np.asarray(e_i, np.float32), np.asarray(e_j, np.float32)], axis=0
    ).astype(bf16)
    ident = np.eye(128, dtype=bf16)
    ones = np.ones((128, 128), dtype=bf16)
    in_maps = []
    for c in range(N_CORES):
        er = np.roll(e, -c * R, axis=0)
        et = np.ascontiguousarray(
            er.reshape(NT, 128, D).transpose(1, 0, 2).reshape(128, NT * D)
        )
        in_maps.append({"e": et, "ident": ident, "ones": ones})

    def _run():
        res = run_bass_kernel_spmd(nc, in_maps, list(range(N_CORES)), trace=_trace)
        _CACHE["last_exec_time_ns"] = res.exec_time_ns
        _CACHE["last_res"] = res

        rowp = np.zeros(TB, np.float64)
        colp = np.zeros(TB, np.float64)
        z2 = np.empty(TB, np.float64)
        pos = np.empty(TB, np.float64)
        for c in range(N_CORES):
            o = res.results[c]
            rows = slice(c * R, (c + 1) * R)
            rowp[rows] = o["rs"].astype(np.float64).T.reshape(-1)
            z2[rows] = o["z2"].astype(np.float64).T.reshape(-1)
            pos[rows] = o["pos"].astype(np.float64).T.reshape(-1)
            # colpart local col j -> global row (c*1024 + j) mod 8192.
            # Tile 39 is never a colsum target (k=32 is row-accum only).
            buf = np.zeros(TB, np.float64)
            buf[: 39 * 128] = o["colp"].astype(np.float64).reshape(-1)[: 39 * 128]
            colp += np.roll(buf, c * R)

        den = rowp + colp - np.exp(z2 / TAU)
        # self-consistency: den ~ 8192*E[exp(sim/2)] in (1.1e3, 6e4);
        # |z2-1| small; |pos| <= ~1. A first-execution race (rare runtime
        # flake) yields garbage here -> caller retries once.
        ok = (
            np.all(np.isfinite(den))
            and den.min() > 1.1e3
            and den.max() < 6e4
            and np.abs(z2 - 1.0).max() < 0.05
            and np.abs(pos).max() < 1.05
        )
        loss = np.mean(np.log(den) - pos / TAU) if ok else np.float64("nan")
        return np.float32(loss), ok

    loss, ok = _run()
    if not ok:
        loss, _ = _run()
    return loss


# revision 23
# speedup vs baseline: 1.1232x; 1.1232x over previous
"""NT-Xent loss Trainium2 kernel, symmetric/circulant variant (8-core SPMD).

sim = Z Z^T is SYMMETRIC: row sums of exp(sim/tau) only need the upper
circulant half. Each core (rotation makes local rows 0..1023 its own)
computes blocks (a, b) for its 8 row-tiles a and b = a+k, k = 0..32 --
tiles 0..39 of the rotated matrix only (2.5MiB in, 40 transposes, and
HALF the exp work of the full-matrix kernel: 264 vs 512 tile-exps).

Per block (a,b): ACT exps the PSUM sim batch into SBUF (bf16) with
accum_out giving this a-row's partial row sums. The PE then multiplies
ones^T @ E (ldweights=ones) accumulating per-COLUMN sums into a PSUM
colacc window per 8-tile octet -- those are the row-sum contributions of
the mirrored blocks (b, a), output per-core as colpart and scatter-added
on the host (np.roll). Blocks at k=32 are computed by BOTH endpoint
cores (the circulant wraps), each contributing row sums only. The k=0
diagonal block's colsums are excluded (its rows are fully in rowp).
Colaccs are DVE-zeroed and accumulated with start=False ONLY: pure
accumulates commute, so the list scheduler cannot corrupt them
(start=True resets interleaved with accumulates DO get misordered).

Host: den_r = rowp_r + colp_r - exp(z2_r/tau);
      loss = mean(log den - pos/tau).
"""

import numpy as np

B = 4096
TB = 2 * B
D = 128
TAU = 0.5
N_CORES = 8
R = TB // N_CORES   # 1024 rows per core
MT = R // 128       # 8 row-tiles owned per core
NT = TB // 128      # 64 row-tiles total
NO = 5              # octets of column tiles held per core (tiles 0..39)
CT = NO * 8         # 40 column tiles
MAGIC = 0x5F3759DF

_CACHE = {}


def _build():
    import concourse.tile as tile
    from concourse import bacc, mybir

    f32 = mybir.dt.float32
    bf16 = mybir.dt.bfloat16
    i32 = mybir.dt.int32
    Exp = mybir.ActivationFunctionType.Exp
    OpAdd = mybir.AluOpType.add
    OpMult = mybir.AluOpType.mult
    OpShr = mybir.AluOpType.arith_shift_right
    OpXor = mybir.AluOpType.bitwise_xor
    AxisX = mybir.AxisListType.X

    nc = bacc.Bacc(
        "TRN2", target_bir_lowering=False, debug=False, num_devices=N_CORES
    )
    # host pre-rotated AND pre-transposed: [128(p), 64(j)*128(d)]
    e_ap = nc.dram_tensor("e", [128, NT * D], bf16, kind="ExternalInput").ap()
    ident_ap = nc.dram_tensor("ident", [128, 128], bf16, kind="ExternalInput").ap()
    ones_ap = nc.dram_tensor("ones", [128, 128], bf16, kind="ExternalInput").ap()
    rs_ap = nc.dram_tensor("rs", [128, MT], f32, kind="ExternalOutput").ap()
    cp_ap = nc.dram_tensor("colp", [1, CT * 128], f32, kind="ExternalOutput").ap()
    pos_ap = nc.dram_tensor("pos", [128, MT], f32, kind="ExternalOutput").ap()
    z2_ap = nc.dram_tensor("z2", [128, MT], f32, kind="ExternalOutput").ap()

    def chunks512(lo, hi):
        # matmul output must not cross a PSUM bank (512 f32 cols)
        j = lo
        while j < hi:
            je = min((j // 512 + 1) * 512, hi)
            yield j, je
            j = je

    with tile.TileContext(nc) as tc:
        with (
            tc.tile_pool(name="xp", bufs=1) as xp,
            tc.tile_pool(name="ztp", bufs=1) as ztp,
            tc.tile_pool(name="small", bufs=1) as sp,
            tc.tile_pool(name="sq", bufs=2) as sqp,
            tc.tile_pool(name="ep", bufs=2) as ep,
            tc.tile_pool(name="ps", bufs=2, space="PSUM") as pp,
            tc.tile_pool(name="ca", bufs=2, space="PSUM") as cap,
        ):
            ident = sp.tile([128, 128], bf16, tag="ident")
            nc.scalar.dma_start(ident[:], ident_ap[:])
            ones = sp.tile([128, 128], bf16, tag="ones")
            nc.scalar.dma_start(ones[:], ones_ap[:])
            dummy = sp.tile([128, 1], f32, tag="dummy")
            nc.scalar.activation(dummy[:], ident[:, 0:1], Exp)

            # Input DMA on the sync queue in octet order; octet 0 split in
            # halves so its prep chain starts ~1us earlier.
            xgs = [
                xp.tile([128, 1024], bf16, tag=f"xg{o}", name=f"xg{o}")
                for o in range(NO)
            ]
            # tiles 4-7 first: o0 blocks [a..7] all need tile 7, and with
            # descending a the first block needs ONLY tile 7.
            nc.sync.dma_start(xgs[0][:, 512:1024], e_ap[:, 512:1024])
            nc.sync.dma_start(xgs[0][:, 0:512], e_ap[:, 0:512])
            for o in range(1, NO):
                nc.sync.dma_start(xgs[o][:], e_ap[:, o * 1024 : (o + 1) * 1024])

            def xtile(t):
                o, j = divmod(t, 8)
                return xgs[o][:, j * 128 : (j + 1) * 128]

            s2 = sp.tile([128, CT], f32, tag="s2")
            inv = sp.tile([128, CT], f32, tag="inv")
            nrt = sp.tile([128, CT], f32, tag="nrt")
            rsparts = sp.tile([128, MT * NO], f32, tag="rsparts")
            rs_t = sp.tile([128, MT], f32, tag="rs")
            pos_t = sp.tile([128, MT], f32, tag="pos")
            z2_t = sp.tile([128, MT], f32, tag="z2")
            inv2 = sp.tile([128, MT], f32, tag="inv2")
            colpart = sp.tile([128, CT * 128], f32, tag="colpart")

            zts = [
                ztp.tile([128, 1024], bf16, tag=f"zt{o}", name=f"zt{o}")
                for o in range(NO)
            ]

            def zttile(t):
                o, j = divmod(t, 8)
                return zts[o][:, j * 128 : (j + 1) * 128]

            def rsqrt(cols):
                s2i = s2[:, cols].bitcast(i32)
                invi = inv[:, cols].bitcast(i32)
                nc.vector.tensor_scalar(
                    out=invi, in0=s2i, scalar1=1, scalar2=-1,
                    op0=OpShr, op1=OpXor,
                )
                nc.vector.tensor_scalar(
                    out=invi, in0=invi, scalar1=MAGIC + 1, scalar2=None, op0=OpAdd
                )
                nr = nrt[:, cols]
                nc.vector.tensor_tensor(nr, inv[:, cols], inv[:, cols], OpMult)
                nc.vector.tensor_tensor(nr, nr, s2[:, cols], OpMult)
                nc.vector.tensor_scalar(
                    out=nr, in0=nr, scalar1=-0.5, scalar2=1.5,
                    op0=OpMult, op1=OpAdd,
                )
                nc.vector.tensor_tensor(inv[:, cols], inv[:, cols], nr, OpMult)

            def prep_dve(o, j0=0, nj=8):
                gcols = slice(o * 8 + j0, o * 8 + j0 + nj)
                sq = sqp.tile([128, nj * 128], f32, tag="sq", name=f"sq{o}_{j0}")
                xs = xgs[o][:, j0 * 128 : (j0 + nj) * 128]
                nc.vector.tensor_tensor(sq[:], xs, xs, OpMult)
                sq3 = sq[:].rearrange("p (j d) -> p j d", d=128)
                nc.vector.tensor_reduce(s2[:, gcols], sq3, axis=AxisX, op=OpAdd)
                rsqrt(gcols)
                for j in range(nj):
                    t = o * 8 + j0 + j
                    nc.vector.tensor_scalar_mul(xtile(t), xtile(t), inv[:, t : t + 1])

            def prep_tp(o, j0, cn, tp, ncopy=1):
                """Transpose cn tiles into zts[o] via the given PSUM scratch
                (a bf16 slice of the NEXT octet's colacc slot, which is idle
                between its flush and memset -- keeps the mm slot rotation
                in the 'ps' pool undisturbed)."""
                for j in range(cn):
                    nc.tensor.transpose(
                        tp[:, j * 128 : (j + 1) * 128], xtile(o * 8 + j0 + j),
                        ident[:],
                    )
                cc = cn // ncopy
                for k in range(0, cn, cc):
                    nc.vector.tensor_copy(
                        zts[o][:, (j0 + k) * 128 : (j0 + k + cc) * 128],
                        tp[:, k * 128 : (k + cc) * 128],
                    )

            def block(o, a, colacc):
                """Process pair-batch (octet o, row-tile a)."""
                blo = max(a, o * 8)
                bhi = min(a + 32, o * 8 + 7)
                nb = bhi - blo + 1
                c0 = (blo - o * 8) * 128
                cw = nb * 128
                mm = pp.tile([128, 1024], f32, tag="ps", name=f"mm{o}_{a}")
                lhsT = zttile(a)
                for j, je in chunks512(c0, c0 + cw):
                    nc.tensor.matmul(mm[:, j:je], lhsT, zts[o][:, j:je])
                et = ep.tile([128, 1024], bf16, tag="e", name=f"e{o}_{a}")
                nc.scalar.activation(
                    et[:, c0 : c0 + cw], mm[:, c0 : c0 + cw], Exp,
                    scale=1.0 / TAU,
                    accum_out=rsparts[:, a * NO + o : a * NO + o + 1],
                )
                # Excluded from colsums: the k==32 tile (o==4 last tile;
                # row-accum on both endpoint cores) and the k==0 diagonal
                # (o==0 first tile; its colsums equal its rowp entry).
                clo = c0 + 128 if o == 0 else 0
                chi = c0 + cw if o < 4 else a * 128
                for j, je in chunks512(clo, chi):
                    nc.tensor.matmul(
                        colacc[:, j:je], ones[:], et[:, j:je],
                        start=False, stop=True,
                    )

            # scheduling anchors (sim ms) -- SPARSE: only to hold next-octet
            # prep out of the current octet's early window. (Dense anchoring
            # rate-limits the real schedule -- measured, do not do it.)
            OSTART = [0.0045, 0.0115, 0.0215, 0.0315, 0.0405]

            # colacc tiles are bf16 [128, 2048] (2 PSUM banks); colsum MMs use
            # the f32 bitcast view [128, 1024]. The bf16 view doubles as the
            # transpose scratch while the slot is idle (flush -> memset gap).
            catile = [None] * NO
            catile[0] = cap.tile([128, 2048], bf16, tag="ca", name="ca0")
            nc.vector.memset(catile[0][:].bitcast(f32), 0.0)
            catile[1] = cap.tile([128, 2048], bf16, tag="ca", name="ca1")

            # octet-0 prep in halves (ramp-critical), tiles 4-7 first. The
            # anchors keep the h1 chain -> h1 transposes -> first blocks
            # ahead of the h0 chain (whose DMA lands ~3us later) and both
            # ahead of prep_dve(1) in the emitted engine order.
            prep_dve(0, 4, 4)
            with tc.tile_wait_until(0.003):
                prep_tp(0, 4, 4, catile[1][:, 0:512], ncopy=2)
            with tc.tile_wait_until(0.0045):
                prep_dve(0, 0, 4)
            with tc.tile_wait_until(0.006):
                prep_tp(0, 0, 4, catile[1][:, 512:1024], ncopy=2)

            for o in range(NO):
                if o == 1:
                    with tc.tile_wait_until(OSTART[1]):
                        nc.vector.tensor_tensor(
                            inv2[:], inv[:, :MT], inv[:, :MT], OpMult
                        )
                        nc.vector.tensor_tensor(
                            z2_t[:], inv2[:], s2[:, :MT], OpMult
                        )
                if o == 4:
                    with tc.tile_wait_until(OSTART[4]):
                        psq = sqp.tile([128, MT * 128], f32, tag="sq", name="psq")
                        nc.vector.tensor_tensor(
                            psq[:], xgs[0][:], xgs[4][:], OpMult
                        )
                        psq3 = psq[:].rearrange("p (m d) -> p m d", d=128)
                        nc.vector.tensor_reduce(pos_t[:], psq3, axis=AxisX, op=OpAdd)
                colacc = catile[o][:].bitcast(f32)
                aorder = range(MT - 1, -1, -1) if o == 0 else range(MT)
                for ai, a in enumerate(aorder):
                    if o < NO - 1 and ai == 0:
                        # for o==0 hold prep_dve(1) clear of the ramp chain
                        with tc.tile_wait_until(0.0065 if o == 0 else OSTART[o]):
                            prep_dve(o + 1)
                    if o < NO - 1 and ai == 4:
                        if catile[o + 1] is None:
                            catile[o + 1] = cap.tile(
                                [128, 2048], bf16, tag="ca", name=f"ca{o + 1}"
                            )
                        w = OSTART[o + 1] - OSTART[o]
                        with tc.tile_wait_until(OSTART[o] + 0.5 * w):
                            prep_tp(o + 1, 0, 4, catile[o + 1][:, 0:512])
                    if o < NO - 1 and ai == 6:
                        w = OSTART[o + 1] - OSTART[o]
                        with tc.tile_wait_until(OSTART[o] + 0.7 * w):
                            prep_tp(o + 1, 4, 4, catile[o + 1][:, 512:1024])
                    if o < NO - 1 and ai == 7:
                        with tc.tile_wait_until(
                            OSTART[o] + 0.85 * (OSTART[o + 1] - OSTART[o])
                        ):
                            nc.vector.memset(catile[o + 1][:].bitcast(f32), 0.0)
                    block(o, a, colacc)
                nc.vector.tensor_copy(
                    colpart[:, o * 1024 : (o + 1) * 1024], colacc
                )

            rsp3 = rsparts[:].rearrange("p (a o) -> p a o", o=NO)
            nc.vector.tensor_reduce(rs_t[:], rsp3, axis=AxisX, op=OpAdd)

            nc.sync.dma_start(rs_ap[:], rs_t[:])
            nc.sync.dma_start(cp_ap[:], colpart[0:1, :])
            nc.sync.dma_start(pos_ap[:], pos_t[:])
            nc.sync.dma_start(z2_ap[:], z2_t[:])

    nc.compile()
    return nc


def _get_nc():
    if "nc" not in _CACHE:
        _CACHE["nc"] = _build()
    return _CACHE["nc"]


def kernel(e_i: np.ndarray, e_j: np.ndarray, _trace: bool = False):
    import ml_dtypes
    from concourse.bass_utils import run_bass_kernel_spmd

    bf16 = ml_dtypes.bfloat16
    nc = _get_nc()
    e = np.concatenate(
        [np.asarray(e_i, np.float32), np.asarray(e_j, np.float32)], axis=0
    ).astype(bf16)
    ident = np.eye(128, dtype=bf16)
    ones = np.ones((128, 128), dtype=bf16)
    in_maps = []
    for c in range(N_CORES):
        er = np.roll(e, -c * R, axis=0)
        et = np.ascontiguousarray(
            er.reshape(NT, 128, D).transpose(1, 0, 2).reshape(128, NT * D)
        )
        in_maps.append({"e": et, "ident": ident, "ones": ones})

    def _run():
        res = run_bass_kernel_spmd(nc, in_maps, list(range(N_CORES)), trace=_trace)
        _CACHE["last_exec_time_ns"] = res.exec_time_ns
        _CACHE["last_res"] = res

        rowp = np.zeros(TB, np.float64)
        colp = np.zeros(TB, np.float64)
        z2 = np.empty(TB, np.float64)
        pos = np.empty(TB, np.float64)
        for c in range(N_CORES):
            o = res.results[c]
            rows = slice(c * R, (c + 1) * R)
            rowp[rows] = o["rs"].astype(np.float64).T.reshape(-1)
            z2[rows] = o["z2"].astype(np.float64).T.reshape(-1)
            pos[rows] = o["pos"].astype(np.float64).T.reshape(-1)
            # colpart local col j -> global row (c*1024 + j) mod 8192.
            # Tile 39 is never a colsum target (k=32 is row-accum only).
            buf = np.zeros(TB, np.float64)
            buf[: 39 * 128] = o["colp"].astype(np.float64).reshape(-1)[: 39 * 128]
            colp += np.roll(buf, c * R)

        den = rowp + colp - np.exp(z2 / TAU)
        # self-consistency: den ~ 8192*E[exp(sim/2)] in (1.1e3, 6e4);
        # |z2-1| small; |pos| <= ~1. A first-execution race (rare runtime
        # flake) yields garbage here -> caller retries once.
        ok = (
            np.all(np.isfinite(den))
            and den.min() > 1.1e3
            and den.max() < 6e4
            and np.abs(z2 - 1.0).max() < 0.05
            and np.abs(pos).max() < 1.05
        )
        loss = np.mean(np.log(den) - pos / TAU) if ok else np.float64("nan")
        return np.float32(loss), ok

    loss, ok = _run()
    if not ok:
        loss, _ = _run()
    return loss


# revision 25
# speedup vs baseline: 1.1244x; 1.0011x over previous
"""NT-Xent loss Trainium2 kernel, symmetric/circulant variant (8-core SPMD).

sim = Z Z^T is SYMMETRIC: row sums of exp(sim/tau) only need the upper
circulant half. Each core (rotation makes local rows 0..1023 its own)
computes blocks (a, b) for its 8 row-tiles a and b = a+k, k = 0..32 --
tiles 0..39 of the rotated matrix only (2.5MiB in, 40 transposes, and
HALF the exp work of the full-matrix kernel: 264 vs 512 tile-exps).

Per block (a,b): ACT exps the PSUM sim batch into SBUF (bf16) with
accum_out giving this a-row's partial row sums. The PE then multiplies
ones^T @ E (ldweights=ones) accumulating per-COLUMN sums into a PSUM
colacc window per 8-tile octet -- those are the row-sum contributions of
the mirrored blocks (b, a), output per-core as colpart and scatter-added
on the host (np.roll). Blocks at k=32 are computed by BOTH endpoint
cores (the circulant wraps), each contributing row sums only. The k=0
diagonal block's colsums are excluded (its rows are fully in rowp).
Colaccs are DVE-zeroed and accumulated with start=False ONLY: pure
accumulates commute, so the list scheduler cannot corrupt them
(start=True resets interleaved with accumulates DO get misordered).

Host: den_r = rowp_r + colp_r - exp(z2_r/tau);
      loss = mean(log den - pos/tau).
"""

import numpy as np

B = 4096
TB = 2 * B
D = 128
TAU = 0.5
N_CORES = 8
R = TB // N_CORES   # 1024 rows per core
MT = R // 128       # 8 row-tiles owned per core
NT = TB // 128      # 64 row-tiles total
NO = 5              # octets of column tiles held per core (tiles 0..39)
CT = NO * 8         # 40 column tiles
MAGIC = 0x5F3759DF

_CACHE = {}


def _build():
    import concourse.tile as tile
    from concourse import bacc, mybir

    f32 = mybir.dt.float32
    bf16 = mybir.dt.bfloat16
    i32 = mybir.dt.int32
    Exp = mybir.ActivationFunctionType.Exp
    OpAdd = mybir.AluOpType.add
    OpMult = mybir.AluOpType.mult
    OpShr = mybir.AluOpType.arith_shift_right
    OpXor = mybir.AluOpType.bitwise_xor
    AxisX = mybir.AxisListType.X

    nc = bacc.Bacc(
        "TRN2", target_bir_lowering=False, debug=False, num_devices=N_CORES
    )
    # host pre-rotated AND pre-transposed: [128(p), 64(j)*128(d)]
    e_ap = nc.dram_tensor("e", [128, NT * D], bf16, kind="ExternalInput").ap()
    ident_ap = nc.dram_tensor("ident", [128, 128], bf16, kind="ExternalInput").ap()
    ones_ap = nc.dram_tensor("ones", [128, 128], bf16, kind="ExternalInput").ap()
    rs_ap = nc.dram_tensor("rs", [128, MT], f32, kind="ExternalOutput").ap()
    cp_ap = nc.dram_tensor("colp", [1, CT * 128], f32, kind="ExternalOutput").ap()
    pos_ap = nc.dram_tensor("pos", [128, MT], f32, kind="ExternalOutput").ap()
    z2_ap = nc.dram_tensor("z2", [128, MT], f32, kind="ExternalOutput").ap()

    def chunks512(lo, hi):
        # matmul output must not cross a PSUM bank (512 f32 cols)
        j = lo
        while j < hi:
            je = min((j // 512 + 1) * 512, hi)
            yield j, je
            j = je

    with tile.TileContext(nc) as tc:
        with (
            tc.tile_pool(name="xp", bufs=1) as xp,
            tc.tile_pool(name="ztp", bufs=1) as ztp,
            tc.tile_pool(name="small", bufs=1) as sp,
            tc.tile_pool(name="sq", bufs=2) as sqp,
            tc.tile_pool(name="ep", bufs=2) as ep,
            tc.tile_pool(name="ps", bufs=2, space="PSUM") as pp,
            tc.tile_pool(name="ca", bufs=2, space="PSUM") as cap,
        ):
            ident = sp.tile([128, 128], bf16, tag="ident")
            nc.scalar.dma_start(ident[:], ident_ap[:])
            ones = sp.tile([128, 128], bf16, tag="ones")
            nc.scalar.dma_start(ones[:], ones_ap[:])
            dummy = sp.tile([128, 1], f32, tag="dummy")
            nc.scalar.activation(dummy[:], ident[:, 0:1], Exp)

            # Input DMA on the sync queue in octet order; octet 0 split in
            # halves so its prep chain starts ~1us earlier.
            xgs = [
                xp.tile([128, 1024], bf16, tag=f"xg{o}", name=f"xg{o}")
                for o in range(NO)
            ]
            # tiles 4-7 first: o0 blocks [a..7] all need tile 7, and with
            # descending a the first block needs ONLY tile 7.
            nc.sync.dma_start(xgs[0][:, 512:1024], e_ap[:, 512:1024])
            nc.sync.dma_start(xgs[0][:, 0:512], e_ap[:, 0:512])
            for o in range(1, NO):
                nc.sync.dma_start(xgs[o][:], e_ap[:, o * 1024 : (o + 1) * 1024])

            def xtile(t):
                o, j = divmod(t, 8)
                return xgs[o][:, j * 128 : (j + 1) * 128]

            s2 = sp.tile([128, CT], f32, tag="s2")
            inv = sp.tile([128, CT], f32, tag="inv")
            nrt = sp.tile([128, CT], f32, tag="nrt")
            rsparts = sp.tile([128, MT * NO], f32, tag="rsparts")
            rs_t = sp.tile([128, MT], f32, tag="rs")
            pos_t = sp.tile([128, MT], f32, tag="pos")
            z2_t = sp.tile([128, MT], f32, tag="z2")
            inv2 = sp.tile([128, MT], f32, tag="inv2")
            colpart = sp.tile([128, CT * 128], f32, tag="colpart")

            zts = [
                ztp.tile([128, 1024], bf16, tag=f"zt{o}", name=f"zt{o}")
                for o in range(NO)
            ]

            def zttile(t):
                o, j = divmod(t, 8)
                return zts[o][:, j * 128 : (j + 1) * 128]

            def rsqrt(cols):
                s2i = s2[:, cols].bitcast(i32)
                invi = inv[:, cols].bitcast(i32)
                nc.vector.tensor_scalar(
                    out=invi, in0=s2i, scalar1=1, scalar2=-1,
                    op0=OpShr, op1=OpXor,
                )
                nc.vector.tensor_scalar(
                    out=invi, in0=invi, scalar1=MAGIC + 1, scalar2=None, op0=OpAdd
                )
                nr = nrt[:, cols]
                nc.vector.tensor_tensor(nr, inv[:, cols], inv[:, cols], OpMult)
                nc.vector.tensor_tensor(nr, nr, s2[:, cols], OpMult)
                nc.vector.tensor_scalar(
                    out=nr, in0=nr, scalar1=-0.5, scalar2=1.5,
                    op0=OpMult, op1=OpAdd,
                )
                nc.vector.tensor_tensor(inv[:, cols], inv[:, cols], nr, OpMult)

            def prep_dve(o, j0=0, nj=8):
                gcols = slice(o * 8 + j0, o * 8 + j0 + nj)
                sq = sqp.tile([128, nj * 128], f32, tag="sq", name=f"sq{o}_{j0}")
                xs = xgs[o][:, j0 * 128 : (j0 + nj) * 128]
                nc.vector.tensor_tensor(sq[:], xs, xs, OpMult)
                sq3 = sq[:].rearrange("p (j d) -> p j d", d=128)
                nc.vector.tensor_reduce(s2[:, gcols], sq3, axis=AxisX, op=OpAdd)
                rsqrt(gcols)
                for j in range(nj):
                    t = o * 8 + j0 + j
                    nc.vector.tensor_scalar_mul(xtile(t), xtile(t), inv[:, t : t + 1])

            def prep_tp(o, j0, cn, tp, ncopy=1):
                """Transpose cn tiles into zts[o] via the given PSUM scratch
                (a bf16 slice of the NEXT octet's colacc slot, which is idle
                between its flush and memset -- keeps the mm slot rotation
                in the 'ps' pool undisturbed)."""
                for j in range(cn):
                    nc.tensor.transpose(
                        tp[:, j * 128 : (j + 1) * 128], xtile(o * 8 + j0 + j),
                        ident[:],
                    )
                cc = cn // ncopy
                for k in range(0, cn, cc):
                    nc.vector.tensor_copy(
                        zts[o][:, (j0 + k) * 128 : (j0 + k + cc) * 128],
                        tp[:, k * 128 : (k + cc) * 128],
                    )

            def block(o, a, colacc):
                """Process pair-batch (octet o, row-tile a)."""
                blo = max(a, o * 8)
                bhi = min(a + 32, o * 8 + 7)
                nb = bhi - blo + 1
                c0 = (blo - o * 8) * 128
                cw = nb * 128
                mm = pp.tile([128, 1024], f32, tag="ps", name=f"mm{o}_{a}")
                lhsT = zttile(a)
                for j, je in chunks512(c0, c0 + cw):
                    nc.tensor.matmul(mm[:, j:je], lhsT, zts[o][:, j:je])
                et = ep.tile([128, 1024], bf16, tag="e", name=f"e{o}_{a}")
                nc.scalar.activation(
                    et[:, c0 : c0 + cw], mm[:, c0 : c0 + cw], Exp,
                    scale=1.0 / TAU,
                    accum_out=rsparts[:, a * NO + o : a * NO + o + 1],
                )
                # Excluded from colsums: the k==32 tile (o==4 last tile;
                # row-accum on both endpoint cores) and the k==0 diagonal
                # (o==0 first tile; its colsums equal its rowp entry).
                clo = c0 + 128 if o == 0 else 0
                chi = c0 + cw if o < 4 else a * 128
                for j, je in chunks512(clo, chi):
                    nc.tensor.matmul(
                        colacc[:, j:je], ones[:], et[:, j:je],
                        start=False, stop=True,
                    )

            # scheduling anchors (sim ms) -- SPARSE: only to hold next-octet
            # prep out of the current octet's early window. (Dense anchoring
            # rate-limits the real schedule -- measured, do not do it.)
            OSTART = [0.0045, 0.0115, 0.0215, 0.0315, 0.0405]

            # colacc tiles are bf16 [128, 2048] (2 PSUM banks); colsum MMs use
            # the f32 bitcast view [128, 1024]. The bf16 view doubles as the
            # transpose scratch while the slot is idle (flush -> memset gap).
            catile = [None] * NO
            catile[0] = cap.tile([128, 2048], bf16, tag="ca", name="ca0")
            nc.vector.memset(catile[0][:].bitcast(f32), 0.0)
            catile[1] = cap.tile([128, 2048], bf16, tag="ca", name="ca1")

            # octet-0 prep in halves (ramp-critical), tiles 4-7 first. The
            # anchors keep the h1 chain -> h1 transposes -> first blocks
            # ahead of the h0 chain (whose DMA lands ~3us later) and both
            # ahead of prep_dve(1) in the emitted engine order.
            prep_dve(0, 4, 4)
            with tc.tile_wait_until(0.003):
                prep_tp(0, 4, 4, catile[1][:, 0:512], ncopy=2)
            with tc.tile_wait_until(0.0045):
                prep_dve(0, 0, 4)
            with tc.tile_wait_until(0.006):
                prep_tp(0, 0, 4, catile[1][:, 512:1024], ncopy=2)

            for o in range(NO):
                if o == 1:
                    with tc.tile_wait_until(OSTART[1]):
                        nc.vector.tensor_tensor(
                            inv2[:], inv[:, :MT], inv[:, :MT], OpMult
                        )
                        nc.vector.tensor_tensor(
                            z2_t[:], inv2[:], s2[:, :MT], OpMult
                        )
                if o == 4:
                    with tc.tile_wait_until(OSTART[4]):
                        psq = sqp.tile([128, MT * 128], f32, tag="sq", name="psq")
                        nc.vector.tensor_tensor(
                            psq[:], xgs[0][:], xgs[4][:], OpMult
                        )
                        psq3 = psq[:].rearrange("p (m d) -> p m d", d=128)
                        nc.vector.tensor_reduce(pos_t[:], psq3, axis=AxisX, op=OpAdd)
                colacc = catile[o][:].bitcast(f32)
                aorder = range(MT - 1, -1, -1) if o == 0 else range(MT)
                for ai, a in enumerate(aorder):
                    if o < NO - 1 and ai == 0:
                        # for o==0 hold prep_dve(1) clear of the ramp chain
                        with tc.tile_wait_until(0.0065 if o == 0 else OSTART[o]):
                            prep_dve(o + 1)
                    if o < NO - 1 and ai == 4:
                        if catile[o + 1] is None:
                            catile[o + 1] = cap.tile(
                                [128, 2048], bf16, tag="ca", name=f"ca{o + 1}"
                            )
                        w = OSTART[o + 1] - OSTART[o]
                        with tc.tile_wait_until(OSTART[o] + 0.5 * w):
                            prep_tp(o + 1, 0, 4, catile[o + 1][:, 0:512])
                    if o < NO - 1 and ai == 6:
                        w = OSTART[o + 1] - OSTART[o]
                        with tc.tile_wait_until(OSTART[o] + 0.7 * w):
                            prep_tp(o + 1, 4, 4, catile[o + 1][:, 512:1024])
                    if o < NO - 1 and ai == 7:
                        with tc.tile_wait_until(
                            OSTART[o] + 0.85 * (OSTART[o + 1] - OSTART[o])
                        ):
                            nc.vector.memset(catile[o + 1][:].bitcast(f32), 0.0)
                    block(o, a, colacc)
                nc.vector.tensor_copy(
                    colpart[:, o * 1024 : (o + 1) * 1024], colacc
                )

            rsp3 = rsparts[:].rearrange("p (a o) -> p a o", o=NO)
            nc.vector.tensor_reduce(rs_t[:], rsp3, axis=AxisX, op=OpAdd)

            nc.sync.dma_start(rs_ap[:], rs_t[:])
            nc.sync.dma_start(cp_ap[:], colpart[0:1, :])
            nc.sync.dma_start(pos_ap[:], pos_t[:])
            nc.sync.dma_start(z2_ap[:], z2_t[:])

    nc.compile()
    return nc


def _get_nc():
    if "nc" not in _CACHE:
        _CACHE["nc"] = _build()
    return _CACHE["nc"]


def kernel(e_i: np.ndarray, e_j: np.ndarray, _trace: bool = False):
    import ml_dtypes
    from concourse.bass_utils import run_bass_kernel_spmd

    bf16 = ml_dtypes.bfloat16
    nc = _get_nc()
    e = np.concatenate(
        [np.asarray(e_i, np.float32), np.asarray(e_j, np.float32)], axis=0
    ).astype(bf16)
    ident = np.eye(128, dtype=bf16)
    ones = np.ones((128, 128), dtype=bf16)
    in_maps = []
    for c in range(N_CORES):
        er = np.roll(e, -c * R, axis=0)
        et = np.ascontiguousarray(
            er.reshape(NT, 128, D).transpose(1, 0, 2).reshape(128, NT * D)
        )
        in_maps.append({"e": et, "ident": ident, "ones": ones})

    def _run():
        res = run_bass_kernel_spmd(nc, in_maps, list(range(N_CORES)), trace=_trace)
        _CACHE["last_exec_time_ns"] = res.exec_time_ns
        _CACHE["last_res"] = res

        rowp = np.zeros(TB, np.float64)
        colp = np.zeros(TB, np.float64)
        z2 = np.empty(TB, np.float64)
        pos = np.empty(TB, np.float64)
        for c in range(N_CORES):
            o = res.results[c]
            rows = slice(c * R, (c + 1) * R)
            rowp[rows] = o["rs"].astype(np.float64).T.reshape(-1)
            z2[rows] = o["z2"].astype(np.float64).T.reshape(-1)
            pos[rows] = o["pos"].astype(np.float64).T.reshape(-1)
            # colpart local col j -> global row (c*1024 + j) mod 8192.
            # Tile 39 is never a colsum target (k=32 is row-accum only).
            buf = np.zeros(TB, np.float64)
            buf[: 39 * 128] = o["colp"].astype(np.float64).reshape(-1)[: 39 * 128]
            colp += np.roll(buf, c * R)

        den = rowp + colp - np.exp(z2 / TAU)
        # self-consistency: den ~ 8192*E[exp(sim/2)] in (1.1e3, 6e4);
        # |z2-1| small; |pos| <= ~1. A first-execution race (rare runtime
        # flake) yields garbage here -> caller retries once.
        ok = (
            np.all(np.isfinite(den))
            and den.min() > 1.1e3
            and den.max() < 6e4
            and np.abs(z2 - 1.0).max() < 0.05
            and np.abs(pos).max() < 1.05
        )
        loss = np.mean(np.log(den) - pos / TAU) if ok else np.float64("nan")
        return np.float32(loss), ok

    loss, ok = _run()
    if not ok:
        loss, _ = _run()
    return loss
